# revision 66
# baseline (speedup 1.0000x reference)
"""Trainium2 Bass kernel for Bahdanau additive attention (nn_Attention).

reference math (per batch b, fp32):
  qa = query @ Wq + bq                  [Tq,U]
  ka = value @ Wk + bk                  [Tv,U]
  scores[q,v] = sum_u Vw[u]*tanh(qa[q,u]+ka[v,u]) + Vb
  weights = softmax(scores - 1e9*~mask)
  result  = layer_norm(query + weights@value)

Sharding: data-parallel over batch B=8 -> one element per NeuronCore.

Core idea (vs materializing the [Tq,Tv,U] feats cube): a separable
sine expansion of tanh,
    tanh(x) ~= sum_k c_k sin(w_k x),   maxerr ~4.6e-3 on |x|<=10.3 at K=7
so
    sum_u wu*tanh(qa+ka) = sum_k [ (wu*c_k*sin(w_k qa))^T cos(w_k ka)
                                  + (wu*c_k*cos(w_k qa))^T sin(w_k ka) ]
which is 2K accumulating PE matmuls over u -- the O(Tq*Tv*U) elementwise
work disappears entirely.  Details:
  * HW ACT Sin only accepts [-pi,pi]; args are range-reduced in fp16 with
    the +1536 magic-rounding trick (rs = round(t)-t, rc = round(t+1/4)-
    t-1/4 for the cos path; the sign flips cancel in the products).
    Mode 0 never exceeds a half period and skips the reduction.
  * basis tiles ride fp16 through DVE 4x tensor_scalar / 2x tensor_tensor
    ops; coefficients wu*c_k fold into one multiply with a shipped tile.
  * softmax exp on ACT with fused row-sum accumulator; masked positions
    get -60000 via a rank-1 PSUM init so exp underflows to exactly 0.
  * the attention row-sum rides a 257th column of the value tile (host-
    packed row sums) so layernorm's mean needs no extra reduction; rstd
    via one Newton step off a 1/(1+v) reciprocal seed (keeps the whole
    tail inside the exp/square ACT table: 2 table loads per iteration).
  * the iteration is split into three software-pipeline stages
    (For_i_pipelined) so DMA latency and the ACT-heavy basis overlap
    across iterations.
"""

import numpy as np

B, TQ, TV, D, U = 8, 128, 256, 256, 128
LN_EPS = 1e-3
N_CORES = 8
NEG_BIG = -60000.0
K = 6
# fp16-exact w_k/(2pi); c refit against these quantized freqs (|x|<=10.3)
FITS = {
    7: ([0.0413818359375, 0.1246337890625, 0.209228515625, 0.295654296875,
         0.383544921875, 0.472900390625, 0.56103515625],
        [1.2400740181078926, 0.33684884771406404, 0.13827895983627533,
         0.05930461427465028, 0.02523871989244504, 0.010490827119195207,
         0.004097818331648479]),
    6: ([0.0416259765625, 0.1253662109375, 0.2105712890625, 0.29736328125,
         0.385498046875, 0.472412109375],
        [1.2396468257787794, 0.33574823613433613, 0.13723773548048457,
         0.05864683810986328, 0.02477608127438866, 0.009783229313782825]),
}
KWP, CK = FITS[K]
MAGIC = 1536.0
SC2PI = float(np.float32(2 * np.pi) * (1.0 - 2.0 ** -20))

# schedule experiment knobs
TWEAKS = {
    "pipeline": True,        # For_i_pipelined 3-stage loop
    "unroll": 6,             # ticks per loop body (amortizes the For_i
                             # all-engine barrier)
    "nbufs": 3,              # staged_num_bufs (SBUF budget)
    "staggered": False,
    "rstd": "newton",        # "newton" (DVE) avoids ln/exp table reloads
    "dve_exp": False,        # softmax exp on DVE (fp16 2^n bit trick).
                             # Removes all act-table loads, but the 13
                             # serial small DVE ops cost ~2us MORE on HW
                             # than the loads (sem/issue overhead the
                             # cost model underestimates) — keep ACT exp.
    "pool_tail": False,      # LN chain on Pool: slower on HW (launch
                             # overhead + cross-engine hops on the
                             # serial tail)
    "dve_copies": False,     # qka PSUM->SBUF copies + etA on DVE
    "npair": 2,              # iterations batched per pipeline stage:
                             # consecutive same-table ACT work shares one
                             # act-table load.  The copies must share
                             # double-width intermediates (column halves)
                             # -- separate per-copy intermediate_tile
                             # allocations deadlock the tile scheduler.
                             # npair=4 (needs const_bufs=1, nbufs=2 for
                             # SBUF) measured WORSE on HW: single-
                             # buffered blobs lose DMA prefetch.
    "const_bufs": 2,
}
LN2 = float(np.log(2.0))

_CACHE = {}


def set_K(k):
    global K, KWP, CK
    K = k
    KWP, CK = FITS[k]


def _groups(cplan):
    """Mode-index groups for ACT/matmul chunked emission.  With the
    pipelined loop, intra-iteration chunking no longer buys overlap, so
    the default is one big group (fewest ACT fixed costs)."""
    h = (K + 1) // 2
    plans = {0: [(0, K)],
             1: [(0, h), (h, K - h)],
             2: [(0, 2), (2, 2), (4, K - 4)],
             3: [(0, 2), (2, K - 2)]}
    return plans.get(cplan, plans[0])


def _build_program(tvc, trivial_ln=True, repeat=0, stage=4, cplan=0, ksrc=0):
    from contextlib import ExitStack
    import concourse.bacc as bacc
    import concourse.tile as tile
    from concourse import mybir

    f32 = mybir.dt.float32
    f16 = mybir.dt.float16
    AF = mybir.ActivationFunctionType
    ALU = mybir.AluOpType

    VA = min(128, tvc)
    VB = tvc - VA
    WA = K * 128          # a-side fused width
    WB = K * tvc          # b-side fused width
    W = WA + WB

    nc = bacc.Bacc("TRN2", target_bir_lowering=False, debug=False)

    ba = nc.dram_tensor("ba", [128, 512], f16, kind="ExternalInput").ap()
    bb = nc.dram_tensor("bb", [128, 256 + 2 * tvc], f16,
                        kind="ExternalInput").ap()
    bk = nc.dram_tensor("bk", [128, K * tvc], f16, kind="ExternalInput").ap()
    bw = nc.dram_tensor("bw", [128, K * 128], f16, kind="ExternalInput").ap()
    bt = nc.dram_tensor("bt", [128, 257 + 257 + 128], f16,
                        kind="ExternalInput").ap()
    if VB:
        bv = nc.dram_tensor("bv", [VB, 257], f16, kind="ExternalInput").ap()
    br = nc.dram_tensor("br", [1, tvc + 128], f16, kind="ExternalInput").ap()
    bs = nc.dram_tensor("bs", [128, 1], f32, kind="ExternalInput").ap()
    if not trivial_ln:
        gamd = nc.dram_tensor("gam", [TQ, D], f32, kind="ExternalInput").ap()
        betd = nc.dram_tensor("bet", [TQ, D], f32, kind="ExternalInput").ap()

    out_res = nc.dram_tensor("out_res", [TQ, D], f32, kind="ExternalOutput").ap()
    out_w = nc.dram_tensor("out_w", [TQ, tvc], f32, kind="ExternalOutput").ap()

    groups = _groups(cplan)

    with tile.TileContext(nc) as tc, ExitStack() as ctx:
        const = ctx.enter_context(tc.tile_pool(
            name="const", bufs=TWEAKS.get("const_bufs", 2)))
        work = ctx.enter_context(tc.tile_pool(name="work", bufs=2))
        psQ = ctx.enter_context(tc.tile_pool(name="psQ", bufs=1, space="PSUM"))
        psK = ctx.enter_context(tc.tile_pool(name="psK", bufs=1, space="PSUM"))
        psS = ctx.enter_context(tc.tile_pool(name="psS", bufs=1, space="PSUM"))
        psT = ctx.enter_context(tc.tile_pool(name="psT", bufs=2, space="PSUM"))
        psA = ctx.enter_context(tc.tile_pool(name="psA", bufs=1, space="PSUM"))

        # NPAIR>1 batches that many independent iterations through each
        # pipeline stage so consecutive same-table ACT work (sins, exps)
        # shares one act-table residency: table loads per iteration drop
        # from 2 to 2/NPAIR.
        NPAIR = TWEAKS.get("npair", 1)

        def stage_a1(pipe, iv, c=0, rs16=None, rc16=None):
            """DMAs + qa/ka matmuls + fp16 range-reduction chain."""
            ba_sb = const.tile([128, 512], f16, name=f"ba_sb{c}")
            nc.sync.dma_start(out=ba_sb[:, :], in_=ba)
            bb_sb = const.tile([128, 256 + 2 * tvc], f16, name=f"bb_sb{c}")
            nc.sync.dma_start(out=bb_sb[:, :], in_=bb)
            bk_sb = const.tile([128, K * tvc], f16, name=f"bk_sb{c}")
            nc.sync.dma_start(out=bk_sb[:, :], in_=bk)
            bs_sb = const.tile([128, 1], f32, name=f"bs_sb{c}")
            nc.gpsimd.dma_start(out=bs_sb[:, :], in_=bs)

            wq0, wq1 = ba_sb[:, 0:128], ba_sb[:, 128:256]
            qT0, qT1 = ba_sb[:, 256:384], ba_sb[:, 384:512]
            wk0, wk1 = bb_sb[:, 0:128], bb_sb[:, 128:256]
            vT0 = bb_sb[:, 256:256 + tvc]
            vT1 = bb_sb[:, 256 + tvc:256 + 2 * tvc]
            kwp = bk_sb[:, :]

            ps_qa = psQ.tile([U, TQ], f32, tag="qa")
            nc.tensor.matmul(ps_qa[:, :], wq0, qT0, start=True, stop=False)
            nc.tensor.matmul(ps_qa[:, :], wq1, qT1, start=False, stop=True)
            ps_ka = psK.tile([U, tvc], f32, tag="ka")
            nc.tensor.matmul(ps_ka[:, :], wk0, vT0, start=True, stop=False)
            nc.tensor.matmul(ps_ka[:, :], wk1, vT1, start=False, stop=True)

            qka = work.tile([U, 128 + tvc], f16, name=f"qka{c}")
            if TWEAKS["dve_copies"]:
                nc.vector.tensor_scalar(qka[:, 0:128], ps_qa[:, :],
                                        bs_sb[:, 0:1], 0.0,
                                        op0=ALU.add, op1=ALU.add)
                nc.vector.tensor_copy(qka[:, 128:128 + tvc], ps_ka[:, :])
            else:
                nc.scalar.activation(qka[:, 0:128], ps_qa[:, :], AF.Identity,
                                     bias=bs_sb[:, 0:1])
                nc.scalar.activation(qka[:, 128:128 + tvc], ps_ka[:, :],
                                     AF.Identity)

            sfx = "" if TWEAKS.get("shared_scratch") else f"_{c}"
            t16 = work.tile([U, W], f16, name=f"t16{sfx}")
            us16 = work.tile([U, W], f16, name=f"us16{sfx}")
            uc16 = work.tile([U, W], f16, name=f"uc16{sfx}")
            ns16 = work.tile([U, W], f16, name=f"ns16{sfx}")
            nc16 = work.tile([U, W], f16, name=f"nc16{sfx}")
            off = c * W   # this copy's column window in the shared tiles

            kw3 = kwp.rearrange("p (k v) -> p k v", k=K)

            def chain(side):
                # mode 0 stays under a half period: sin arg is t directly
                # (+sin/+cos there vs -sin/-cos elsewhere cancels in the
                # products); modes 1..K-1 get the rounding chain.
                if side == 0:
                    w, base = 128, 0
                    src = qka[:, 0:128]
                    in1f = kw3[:, :, 0:128]
                else:
                    w, base = tvc, WA
                    src = qka[:, 128:128 + tvc]
                    in1f = kw3
                sl = slice(base + w, base + K * w)          # work tiles
                so = slice(off + base + w, off + base + K * w)  # shared out
                s0 = slice(off + base, off + base + w)
                r3 = "p (k j) -> p k j"
                nc.vector.tensor_tensor(
                    rs16[:, s0].rearrange(r3, k=1),
                    src.unsqueeze(1).broadcast_to([U, 1, w]),
                    in1f[:, 0:1, :], op=ALU.mult)
                nc.vector.tensor_scalar(rc16[:, s0], rs16[:, s0], 0.25, 0.0,
                                        op0=ALU.add, op1=ALU.add)
                nc.vector.tensor_tensor(
                    t16[:, sl].rearrange(r3, k=K - 1),
                    src.unsqueeze(1).broadcast_to([U, K - 1, w]),
                    in1f[:, 1:K, :], op=ALU.mult)
                nc.vector.tensor_scalar(us16[:, sl], t16[:, sl], MAGIC, 0.0,
                                        op0=ALU.add, op1=ALU.add)
                nc.vector.tensor_scalar(ns16[:, sl], us16[:, sl], -MAGIC,
                                        0.0, op0=ALU.add, op1=ALU.add)
                nc.vector.tensor_tensor(rs16[:, so], ns16[:, sl],
                                        t16[:, sl], op=ALU.subtract)
                nc.vector.tensor_scalar(uc16[:, sl], t16[:, sl], 0.25,
                                        MAGIC, op0=ALU.add, op1=ALU.add)
                nc.vector.tensor_scalar(nc16[:, sl], uc16[:, sl],
                                        -(MAGIC + 0.25), 0.0,
                                        op0=ALU.add, op1=ALU.add)
                nc.vector.tensor_tensor(rc16[:, so], nc16[:, sl],
                                        t16[:, sl], op=ALU.subtract)

            chain(0)
            chain(1)
            return (rs16, rc16)

        def stage_a2(pipe, iv, prev, c=0, outs=None):
            """ACT sins + coefficient multiplies."""
            rs16, rc16 = prev
            sta, cta, sbar, cbar = outs
            off, offa = c * W, c * WA
            bw_sb = const.tile([128, K * 128], f16, name=f"bw_sb{c}")
            nc.gpsimd.dma_start(out=bw_sb[:, :], in_=bw)

            for (k0, kn) in groups:
                a0, a1 = k0 * 128, (k0 + kn) * 128
                b0 = WA + k0 * tvc
                b1 = WA + (k0 + kn) * tvc
                nc.scalar.activation(sbar[:, off + a0:off + a1],
                                     rs16[:, off + a0:off + a1],
                                     AF.Sin, scale=SC2PI)
                nc.scalar.activation(cbar[:, off + a0:off + a1],
                                     rc16[:, off + a0:off + a1],
                                     AF.Sin, scale=SC2PI)
                nc.scalar.activation(sbar[:, off + b0:off + b1],
                                     rs16[:, off + b0:off + b1],
                                     AF.Sin, scale=SC2PI)
                nc.scalar.activation(cbar[:, off + b0:off + b1],
                                     rc16[:, off + b0:off + b1],
                                     AF.Sin, scale=SC2PI)
                nc.vector.tensor_tensor(sta[:, offa + a0:offa + a1],
                                        sbar[:, off + a0:off + a1],
                                        bw_sb[:, a0:a1], op=ALU.mult)
                nc.vector.tensor_tensor(cta[:, offa + a0:offa + a1],
                                        cbar[:, off + a0:off + a1],
                                        bw_sb[:, a0:a1], op=ALU.mult)

        def stage_b(pipe, iv, prev, c=0):
            """Score matmuls + softmax + attention + layernorm + outputs."""
            sta, cta, sbar, cbar = prev
            boff, boffa = c * W, c * WA
            bt_sb = const.tile([128, 257 + 257 + 128], f16, name=f"bt_sb{c}")
            nc.sync.dma_start(out=bt_sb[:, :], in_=bt)
            br_sb = const.tile([1, tvc + 128], f16, name=f"br_sb{c}")
            nc.gpsimd.dma_start(out=br_sb[:, :], in_=br)
            if VB:
                bv_sb = const.tile([VB, 257], f16, name=f"bv_sb{c}")
                nc.gpsimd.dma_start(out=bv_sb[:, :], in_=bv)
            if not trivial_ln:
                gam_sb = const.tile([TQ, D], f32, name="gam_sb")
                nc.sync.dma_start(out=gam_sb[:, :], in_=gamd)
                bet_sb = const.tile([TQ, D], f32, name="bet_sb")
                nc.sync.dma_start(out=bet_sb[:, :], in_=betd)

            vca = bt_sb[:, 0:257]
            qnp = bt_sb[:, 257:514]
            iden = bt_sb[:, 514:642]
            maskrow = br_sb[0:1, 0:tvc]
            onesq = br_sb[0:1, tvc:tvc + 128]

            ps_sc = psS.tile([TQ, tvc], f32, tag="sc")
            nc.tensor.matmul(ps_sc[:, :], onesq, maskrow,
                             start=True, stop=False)
            nmm = 2 * K
            i = 0
            for k in range(K):
                ca, cb = boffa + k * 128, boff + WA + k * tvc
                i += 1
                nc.tensor.matmul(ps_sc[:, :], sta[:, ca:ca + 128],
                                 cbar[:, cb:cb + tvc],
                                 start=False, stop=False)
                i += 1
                nc.tensor.matmul(ps_sc[:, :], cta[:, ca:ca + 128],
                                 sbar[:, cb:cb + tvc],
                                 start=False, stop=(i == nmm))

            exp16 = work.tile([TQ, tvc], f16, name="exp16")
            den = work.tile([TQ, 1], f32, name="den")
            if TWEAKS["dve_exp"]:
                # exp(s) = 2^round(s/ln2) * e^(ln2*r), r = s/ln2 - round
                # 2^n built by fp16->int16 convert + bitcast (n clamped to
                # [-15,14]; -15 gives exactly 0 for masked positions).
                i16 = mybir.dt.int16
                F32MAGIC = 12582912.0   # 1.5*2^23: fp32 round-to-int
                t2 = work.tile([TQ, tvc], f32, name="e_t2")
                nc.vector.tensor_scalar(t2[:, :], ps_sc[:, :], 1.0 / LN2,
                                        -100.0, op0=ALU.mult, op1=ALU.max)
                u2 = work.tile([TQ, tvc], f32, name="e_u2")
                nc.vector.tensor_scalar(u2[:, :], t2[:, :], F32MAGIC, 0.0,
                                        op0=ALU.add, op1=ALU.add)
                n2 = work.tile([TQ, tvc], f32, name="e_n2")
                nc.vector.tensor_scalar(n2[:, :], u2[:, :], -F32MAGIC, 0.0,
                                        op0=ALU.add, op1=ALU.add)
                r2 = work.tile([TQ, tvc], f16, name="e_r2")
                nc.vector.tensor_tensor(r2[:, :], t2[:, :], n2[:, :],
                                        op=ALU.subtract)
                # Horner: e^(ln2 r) = 1 + z(1 + z/2(1 + z/3)), z = ln2*r
                h1 = work.tile([TQ, tvc], f16, name="e_h1")
                nc.vector.tensor_scalar(h1[:, :], r2[:, :], LN2 / 3.0, 1.0,
                                        op0=ALU.mult, op1=ALU.add)
                m1 = work.tile([TQ, tvc], f16, name="e_m1")
                nc.vector.tensor_tensor(m1[:, :], h1[:, :], r2[:, :],
                                        op=ALU.mult)
                h2 = work.tile([TQ, tvc], f16, name="e_h2")
                nc.vector.tensor_scalar(h2[:, :], m1[:, :], LN2 / 2.0, 1.0,
                                        op0=ALU.mult, op1=ALU.add)
                m2e = work.tile([TQ, tvc], f16, name="e_m2")
                nc.vector.tensor_tensor(m2e[:, :], h2[:, :], r2[:, :],
                                        op=ALU.mult)
                pol = work.tile([TQ, tvc], f16, name="e_pol")
                nc.vector.tensor_scalar(pol[:, :], m2e[:, :], LN2, 1.0,
                                        op0=ALU.mult, op1=ALU.add)
                # (clamp(n,-15)+15)*1024, exact in fp16, then convert+cast
                nb = work.tile([TQ, tvc], f32, name="e_nb")
                nc.vector.tensor_scalar(nb[:, :], n2[:, :], 1.0, -15.0,
                                        op0=ALU.mult, op1=ALU.max)
                na = work.tile([TQ, tvc], f16, name="e_na")
                nc.vector.tensor_scalar(na[:, :], nb[:, :], 1024.0,
                                        15360.0, op0=ALU.mult, op1=ALU.add)
                ni = work.tile([TQ, tvc], i16, name="e_ni")
                nc.vector.tensor_copy(ni[:, :], na[:, :])
                nc.vector.tensor_tensor(exp16[:, :], pol[:, :],
                                        ni[:, :].bitcast(f16), op=ALU.mult)
                nc.vector.reduce_sum(den[:, :], exp16[:, :],
                                     axis=mybir.AxisListType.X)
            else:
                nc.scalar.activation(exp16[:, :], ps_sc[:, :], AF.Exp,
                                     accum_out=den[:, 0:1])
            rinv = work.tile([TQ, 1], f32, name="rinv")
            nc.vector.reciprocal(rinv[:, :], den[:, :])

            # normalized weights -> DRAM (off critical path, on Pool)
            w_sb = work.tile([TQ, tvc], f32, name="w_sb")
            nc.gpsimd.tensor_scalar(w_sb[:, :], exp16[:, :], rinv[:, 0:1],
                                    0.0, op0=ALU.mult, op1=ALU.add)
            nc.gpsimd.dma_start(out=out_w, in_=w_sb[:, :])

            ps_tA = psT.tile([128, 128], f16, tag="trA", name="trA")
            nc.tensor.transpose(ps_tA[0:VA, 0:TQ], exp16[:, 0:VA], iden)
            etA = work.tile([VA, TQ], f16, name="etA")
            if TWEAKS["dve_copies"]:
                nc.vector.tensor_copy(etA[:, :], ps_tA[0:VA, 0:TQ])
            else:
                nc.scalar.copy(etA[:, :], ps_tA[0:VA, 0:TQ])
            if VB:
                ps_tB = psT.tile([128, 128], f16, tag="trB", name="trB")
                nc.tensor.transpose(ps_tB[0:VB, 0:TQ], exp16[:, VA:tvc], iden)
                etB = work.tile([VB, TQ], f16, name="etB")
                nc.vector.tensor_copy(etB[:, :], ps_tB[0:VB, 0:TQ])

            ps_at = psA.tile([TQ, 257], f32, tag="at")
            nc.tensor.matmul(ps_at[:, :], etA[:, :], vca[0:VA, :],
                             start=True, stop=not VB)
            if VB:
                nc.tensor.matmul(ps_at[:, :], etB[:, :], bv_sb[:, :],
                                 start=False, stop=True)

            # residual + layernorm; x[:,256] is the row sum (vc trick)
            x_sb = work.tile([TQ, 257], f32, name="x_sb")
            nc.vector.scalar_tensor_tensor(x_sb[:, :], ps_at[:, :],
                                           rinv[:, 0:1], qnp,
                                           op0=ALU.mult, op1=ALU.add)
            ln_eng = nc.gpsimd if TWEAKS["pool_tail"] else nc.vector
            negmu = work.tile([TQ, 1], f32, name="negmu")
            ln_eng.tensor_scalar(negmu[:, :], x_sb[:, 256:257], -1.0 / D,
                                 0.0, op0=ALU.mult, op1=ALU.add)
            m2 = work.tile([TQ, 1], f32, name="m2")
            ln_eng.tensor_scalar(m2[:, :], x_sb[:, 256:257],
                                 x_sb[:, 256:257], 1.0 / (D * D),
                                 op0=ALU.mult, op1=ALU.mult)
            xsq = work.tile([TQ, D], f16, name="xsq")
            sqs = work.tile([TQ, 1], f32, name="sqs")
            nc.scalar.activation(xsq[:, :], x_sb[:, 0:256], AF.Square,
                                 accum_out=sqs[:, 0:1])
            sqv = work.tile([TQ, 1], f32, name="sqv")
            ln_eng.tensor_scalar(sqv[:, :], sqs[:, :], 1.0 / D, LN_EPS,
                                 op0=ALU.mult, op1=ALU.add)
            veps = work.tile([TQ, 1], f32, name="veps")
            ln_eng.tensor_sub(veps[:, :], sqv[:, :], m2[:, :])
            rstd = work.tile([TQ, 1], f32, name="rstd")
            if TWEAKS["rstd"] == "lnexp":
                lnv = work.tile([TQ, 1], f32, name="lnv")
                nc.scalar.activation(lnv[:, :], veps[:, :], AF.Ln)
                nc.scalar.activation(rstd[:, :], lnv[:, :], AF.Exp,
                                     scale=-0.5)
            else:
                squ = work.tile([TQ, 1], f32, name="squ")
                ln_eng.tensor_scalar(squ[:, :], sqs[:, :], 1.0 / D,
                                     LN_EPS + 1.0, op0=ALU.mult,
                                     op1=ALU.add)
                u_t = work.tile([TQ, 1], f32, name="u_t")
                ln_eng.tensor_sub(u_t[:, :], squ[:, :], m2[:, :])
                w_t = work.tile([TQ, 1], f32, name="w_t")
                nc.vector.reciprocal(w_t[:, :], u_t[:, :])
                b_t = work.tile([TQ, 1], f32, name="b_t")
                nc.vector.scalar_tensor_tensor(b_t[:, :], w_t[:, :],
                                               w_t[:, 0:1], veps[:, :],
                                               op0=ALU.mult, op1=ALU.mult)
                tt_t = work.tile([TQ, 1], f32, name="tt_t")
                ln_eng.tensor_scalar(tt_t[:, :], b_t[:, :], -4.0, 3.0,
                                     op0=ALU.mult, op1=ALU.add)
                ln_eng.tensor_mul(rstd[:, :], w_t[:, :], tt_t[:, :])
            res_sb = work.tile([TQ, D], f32, name="res_sb")
            ln_eng.tensor_scalar(res_sb[:, :], x_sb[:, 0:256],
                                 negmu[:, 0:1], rstd[:, 0:1],
                                 op0=ALU.add, op1=ALU.mult)
            if not trivial_ln:
                r2 = work.tile([TQ, D], f32, name="r2")
                nc.vector.tensor_mul(r2[:, :], res_sb[:, :], gam_sb[:, :])
                nc.vector.tensor_add(r2[:, :], r2[:, :], bet_sb[:, :])
                res_sb = r2
            nc.gpsimd.dma_start(out=out_res, in_=res_sb[:, :])

        class _SeqPipe:
            def __init__(self, pool):
                self.pool = pool
                self.n = 0

            def intermediate_tile(self, shape, dtype):
                self.n += 1
                return self.pool.tile(shape, dtype, name=f"pi{self.n}")

        # pair-batched stage wrappers: same-table ACT work of the NPAIR
        # copies runs back-to-back, sharing one act-table load.  The
        # copies share double-width intermediates (column halves) so the
        # per-stage intermediate count matches the single-copy case.
        def p_a1(pipe, iv):
            rs16 = pipe.intermediate_tile([U, NPAIR * W], f16)
            rc16 = pipe.intermediate_tile([U, NPAIR * W], f16)
            for c in range(NPAIR):
                stage_a1(pipe, iv, c=c, rs16=rs16, rc16=rc16)
            return (rs16, rc16)

        def p_a2(pipe, iv, prev):
            sta = pipe.intermediate_tile([U, NPAIR * WA], f16)
            cta = pipe.intermediate_tile([U, NPAIR * WA], f16)
            sbar = pipe.intermediate_tile([U, NPAIR * W], f16)
            cbar = pipe.intermediate_tile([U, NPAIR * W], f16)
            outs = (sta, cta, sbar, cbar)
            for c in range(NPAIR):
                stage_a2(pipe, iv, prev, c=c, outs=outs)
            return outs

        def p_b(pipe, iv, prev):
            for c in range(NPAIR):
                stage_b(pipe, iv, prev, c=c)

        stages = [p_a1, p_a2, p_b]
        hints = (mybir.EngineType.PE, mybir.EngineType.DVE,
                 mybir.EngineType.Activation, mybir.EngineType.SP,
                 mybir.EngineType.Pool)
        if repeat and TWEAKS["pipeline"]:
            assert repeat % NPAIR == 0, (repeat, NPAIR)
            tc.For_i_pipelined(stages, 0, repeat // NPAIR, 1,
                               unroll=TWEAKS["unroll"],
                               staged_num_bufs=TWEAKS["nbufs"],
                               staggered_reset=TWEAKS["staggered"],
                               hint_engines=hints)
        def seq_once(p):
            rs16 = p.intermediate_tile([U, W], f16)
            rc16 = p.intermediate_tile([U, W], f16)
            stage_a1(p, 0, c=0, rs16=rs16, rc16=rc16)
            outs = (p.intermediate_tile([U, WA], f16),
                    p.intermediate_tile([U, WA], f16),
                    p.intermediate_tile([U, W], f16),
                    p.intermediate_tile([U, W], f16))
            stage_a2(p, 0, (rs16, rc16), c=0, outs=outs)
            stage_b(p, 0, outs, c=0)

        if repeat and TWEAKS["pipeline"]:
            pass
        elif repeat:
            with tc.For_i(0, repeat, 1, hint_engines=hints,
                          staggered_reset=TWEAKS["staggered"]):
                seq_once(_SeqPipe(work))
        else:
            seq_once(_SeqPipe(work))

    nc.compile()
    return nc


def _plan(v_mask):
    counts = v_mask.sum(axis=1)
    tvc = int(-(-max(int(counts.max()), 8) // 8) * 8)
    idxs = [np.where(v_mask[b])[0] for b in range(v_mask.shape[0])]
    return tvc, idxs


def _host_prep(query, value, v_mask, Wq_w, Wq_b, Wk_w, Wk_b, V_w, ln_gamma,
               ln_beta, tvc, idxs, trivial_ln):
    f16 = np.float16
    f32 = np.float32
    VA = min(128, tvc)
    VB = tvc - VA

    wq16 = Wq_w.astype(f16)
    wk16 = Wk_w.astype(f16)
    kwp = np.asarray(KWP, f16)
    ck = np.asarray(CK, f32)
    vw = V_w.astype(f32).reshape(U)

    # [u, K*tvc] : kwp[k] everywhere; a-side slices [k*tvc : k*tvc+128]
    kw_tile = np.broadcast_to(kwp[None, :, None],
                              (128, K, tvc)).reshape(128, K * tvc)
    # [u, K*128] : vw[u]*ck[k]
    wc_tile = (vw[:, None, None] * ck[None, :, None]
               ).astype(f16).repeat(128, axis=2).reshape(128, K * 128)

    in_maps = []
    for b in range(B):
        q = query[b].astype(f32)
        idx = idxs[b]
        cnt = len(idx)
        vcomp = np.zeros((tvc, D), f32)
        vcomp[:cnt] = value[b][idx]
        vT = vcomp.T.astype(f16)
        qT = q.T.astype(f16)

        ba = np.zeros((128, 512), f16)
        ba[:, 0:128] = wq16[0:128]
        ba[:, 128:256] = wq16[128:256]
        ba[:, 256:384] = qT[0:128]
        ba[:, 384:512] = qT[128:256]

        bb = np.zeros((128, 256 + 2 * tvc), f16)
        bb[:, 0:128] = wk16[0:128]
        bb[:, 128:256] = wk16[128:256]
        bb[:, 256:256 + tvc] = vT[0:128]
        bb[:, 256 + tvc:256 + 2 * tvc] = vT[128:256]

        vc16 = vcomp.astype(f16)
        vcp = np.zeros((tvc, 257), f16)
        vcp[:, 0:256] = vc16
        vcp[:, 256] = vc16.astype(f32).sum(axis=1).astype(f16)
        qn16 = q.astype(f16)
        qnp = np.zeros((TQ, 257), f16)
        qnp[:, 0:256] = qn16
        qnp[:, 256] = qn16.astype(f32).sum(axis=1).astype(f16)

        bt = np.zeros((128, 642), f16)
        bt[:, 0:257] = vcp[0:VA]
        bt[:, 257:514] = qnp
        bt[:, 514:642] = np.eye(128, dtype=f16)

        br = np.zeros((1, tvc + 128), f16)
        maskr = np.full((tvc,), NEG_BIG, f32)
        maskr[:cnt] = 0.0
        br[0, 0:tvc] = maskr.astype(f16)
        br[0, tvc:tvc + 128] = 1.0

        bs = (Wq_b.astype(f32) + Wk_b.astype(f32)).reshape(128, 1)

        m = {"ba": ba, "bb": bb, "bk": kw_tile.astype(f16), "bw": wc_tile,
             "bt": bt, "br": br, "bs": bs}
        if VB:
            m["bv"] = vcp[VA:tvc]
        if not trivial_ln:
            m["gam"] = np.broadcast_to(ln_gamma.astype(f32), (TQ, D)).copy()
            m["bet"] = np.broadcast_to(ln_beta.astype(f32), (TQ, D)).copy()
        in_maps.append(m)
    return in_maps


def kernel(query, value, v_mask, Wq_w, Wq_b, Wk_w, Wk_b, V_w, V_b, ln_gamma,
           ln_beta):
    from concourse.bass_utils import run_bass_kernel_spmd

    query = np.asarray(query, np.float32)
    value = np.asarray(value, np.float32)
    v_mask = np.asarray(v_mask, bool)
    tvc, idxs = _plan(v_mask)
    trivial_ln = bool(np.all(np.asarray(ln_gamma) == 1.0)
                      and np.all(np.asarray(ln_beta) == 0.0))
    key = (tvc, trivial_ln)
    if key not in _CACHE:
        _CACHE[key] = _build_program(tvc, trivial_ln)
    nc = _CACHE[key]
    in_maps = _host_prep(query, value, v_mask, Wq_w, Wq_b, Wk_w, Wk_b, V_w,
                         ln_gamma, ln_beta, tvc, idxs, trivial_ln)
    # V_b shifts all scores equally; softmax is invariant, so it is a no-op,
    # but fold it into the mask row anyway for fidelity.
    vb = float(np.asarray(V_b).reshape(-1)[0])
    if vb != 0.0:
        for m in in_maps:
            row = m["br"].astype(np.float32)
            row[0, :tvc] += vb
            m["br"] = row.astype(np.float16)
    res = run_bass_kernel_spmd(nc, in_maps, core_ids=list(range(N_CORES)))
    result = np.stack([res.results[b]["out_res"] for b in range(B)])
    weights = np.zeros((B, TQ, TV), np.float32)
    for b in range(B):
        cnt = len(idxs[b])
        weights[b][:, idxs[b]] = res.results[b]["out_w"][:, :cnt]
    return result.astype(np.float32), weights
